# revision 1
# baseline (speedup 1.0000x reference)
"""CNLP (common-neighbor link prediction) kernel for Trainium2, 8 NeuronCores.

Reference computation (per query edge e = (i, j)):
    cn  = adj[i] * adj[j]                      # common-neighbor indicator [N]
    xcn = cn @ x                               # sum of common-neighbor feats
    xij = relu(x[i]*x[j] @ Wa.T + ba) @ Wb.T + bb
    hcn = (relu->relu->lin) 3-layer MLP on xcn
    out = (hcn * beta + xij) @ Wl.T + bl       # [E, 1]

Sharding: edges (E=8192) split 8 x 1024 across cores; adj/x/weights replicated.

Device strategy per core (1024 edges in 2 blocks of 512):
  - adj is binary -> uploaded as fp8e4 (exact), padded N->10240.
  - Per (block, n-quarter): ONE gpsimd dma_gather(transpose=True) pulls the
    adjacency rows for idx list [e0 || e1] (1024 idxs x 2560 B) into
    [128, 10, 2, 1024] fp8; DVE ANDs the two halves in 16-bit 2x mode -> cn.
  - Big matmul is FLIPPED vs the e-major formulation: stationary = fp8 x
    table (host-permuted to match the 16-bit gather interleave), moving =
    cn slices; PSUM accumulates xcn^T feature-major [128f, 512e] directly
    (no PE transposes).  DOUBLE_ROW uses fp8 DoubleRow perf mode (2 k-tiles
    = adjacent word-chunks) for 2x PE throughput.
  - MLPs run feature-major with fp32 weights/activations read as float32r
    (fp22) -> 1 cycle/row instead of 4.  xij path gathers bf16 x rows.
"""

import numpy as np
import ml_dtypes

import concourse.bacc as bacc
import concourse.tile as tile
import concourse.mybir as mybir
from concourse.bass_utils import run_bass_kernel_spmd

BF16 = mybir.dt.bfloat16
FP32 = mybir.dt.float32
FP32R = mybir.dt.float32r
FP8 = mybir.dt.float8e4
I16 = mybir.dt.int16
AF = mybir.ActivationFunctionType
BF16_NP = ml_dtypes.bfloat16
FP8_NP = ml_dtypes.float8_e4m3

N_CORES = 8
N, E, D, H = 10000, 8192, 256, 256
NPAD = 10240                      # n padded to a multiple of 256
EC = E // N_CORES                 # 1024 edges per core
EB = 512                          # edges per block
NB = EC // EB                     # 2 blocks
# adjacency rows extended with the bf16 x row: [adj fp8 10240B | x bf16 512B];
# gathered in four splits (all %256), the last carrying the x row
ROWB = NPAD + 2 * D               # 10752 bytes per extended row
NSPLIT = 4
QOFF = (0, 2560, 5120, 7680)      # byte offset of each split
QELEM = (2560, 2560, 2560, 3072)  # gather elem bytes
QC2 = (10, 10, 10, 10)            # adjacency word-chunks (256 nodes) per split
GBASE = (0, 10, 20, 30)           # global word-chunk base per split
NGC = 40                          # total word-chunks (10240 nodes)

DOUBLE_ROW = True


def build_program():
    nc = bacc.Bacc("TRN2", target_bir_lowering=False, debug=False,
                   enable_asserts=False, num_devices=N_CORES)

    adjx = nc.dram_tensor("adjx", [N, ROWB], FP8, kind="ExternalInput")
    x8t_d = nc.dram_tensor("x8t", [128, NGC * 2 * 2 * 128], FP8,
                           kind="ExternalInput")
    # gather idx (raw node ids), 512-idx wraps ordered (b, src); shared by
    # the adjacency gathers and the (unpermuted) xh gathers
    idxg_d = nc.dram_tensor("idxg", [128, NB * 2 * EB // 16], I16,
                            kind="ExternalInput")
    wts_d = {nm: nc.dram_tensor(nm, [D, H], FP32, kind="ExternalInput")
             for nm in ("wat", "wbt", "w1t", "w2t", "w3t")}
    wlt_d = nc.dram_tensor("wlt", [H, 1], FP32, kind="ExternalInput")
    bias_d = {nm: nc.dram_tensor(nm, [2, 128, 1], FP32, kind="ExternalInput")
              for nm in ("ba", "bb", "b1", "b2", "b3")}
    bl_d = nc.dram_tensor("bl", [1, 1], FP32, kind="ExternalInput")
    beta_d = nc.dram_tensor("beta", [128, 1], FP32, kind="ExternalInput")
    out_d = nc.dram_tensor("out", [1, EC], FP32, kind="ExternalOutput")

    with tile.TileContext(nc) as tc:
        with (
            tc.tile_pool(name="const", bufs=1) as constp,
            tc.tile_pool(name="gath", bufs=4) as gathp,
            tc.tile_pool(name="acts", bufs=6) as actp,
            tc.tile_pool(name="px", bufs=4, space="PSUM") as pxp,
            tc.tile_pool(name="pm", bufs=2, space="PSUM") as pmp,
            tc.tile_pool(name="po", bufs=2, space="PSUM") as pop,
        ):
            # ---- persistent loads -------------------------------------
            # idx tiles FIRST (gathers wait on them; HWDGE is FIFO)
            idxg_sb = constp.tile([128, NB * 2 * EB // 16], I16)
            nc.sync.dma_start(idxg_sb[:], idxg_d[:])

            # fp8 x table, split in 4 loads so early matmuls start sooner
            x8t_sb = constp.tile([128, NGC, 2, 2, 128], FP8)
            qsz = (NGC // 4) * 2 * 2 * 128
            for q in range(4):
                nc.sync.dma_start(
                    x8t_sb[:, q * (NGC // 4):(q + 1) * (NGC // 4), :, :, :]
                    .rearrange("p c t f g -> p (c t f g)"),
                    x8t_d[:, q * qsz:(q + 1) * qsz])

            # weights: DMA fp32 staging -> scalar-copy to fp32r (the BIR
            # verifier requires fp32r matmul operands to be fp32r-rounded)
            w_sb = {}
            for nm, d in wts_d.items():
                stg = constp.tile([128, 2, H], FP32, tag="wstg", bufs=2,
                                  name=f"ws_{nm}")
                nc.sync.dma_start(stg[:], d[:].rearrange("(k p) h -> p k h", p=128))
                t = constp.tile([128, 2, H], FP32R, tag=f"w_{nm}")
                nc.scalar.activation(t[:], stg[:], AF.Copy)
                w_sb[nm] = t
            wlt_stg = constp.tile([128, 2, 1], FP32)
            nc.sync.dma_start(wlt_stg[:], wlt_d[:].rearrange("(k p) o -> p k o", p=128))
            wlt_sb = constp.tile([128, 2, 1], FP32R)
            nc.scalar.activation(wlt_sb[:], wlt_stg[:], AF.Copy)
            b_sb = {}
            for nm, d in bias_d.items():
                t = constp.tile([128, 2, 1], FP32, tag=f"b_{nm}")
                nc.sync.dma_start(t[:], d[:].rearrange("t p o -> p t o"))
                b_sb[nm] = t
            bl_sb = constp.tile([1, 1], FP32)
            nc.sync.dma_start(bl_sb[:], bl_d[:])
            beta_sb = constp.tile([128, 1], FP32)
            nc.sync.dma_start(beta_sb[:], beta_d[:])

            out_sb = constp.tile([1, EC], FP32)

            # MLP layer, feature-major fp32r (fp22 reads, 1 cyc/row), 512 edges
            def lin_h(src, wname, bname, relu, dst):
                w, bias = w_sb[wname], b_sb[bname]
                for t in range(2):
                    pm = pmp.tile([128, EB], FP32, tag="pm")
                    for k in range(2):
                        nc.tensor.matmul(
                            pm[:], w[:, k, t * 128:(t + 1) * 128],
                            src[:, k, :], start=(k == 0), stop=(k == 1))
                    dsl = dst[:, t, :]
                    if t % 2 == 0:
                        nc.scalar.activation(
                            dsl, pm[:], AF.Relu if relu else AF.Identity,
                            bias=bias[:, t, :])
                    elif relu:
                        nc.vector.tensor_scalar(
                            dsl, pm[:], bias[:, t, :], 0.0,
                            mybir.AluOpType.add, mybir.AluOpType.max)
                    else:
                        nc.vector.tensor_scalar_add(dsl, pm[:], bias[:, t, :])
                return dst

            def mlp_block(b, xcn_sb, xiT, xjT):
                pT = actp.tile([128, 2, EB], FP32R, tag="act")
                nc.vector.tensor_mul(pT[:], xiT, xjT)
                u = lin_h(pT, "wat", "ba", True,
                          actp.tile([128, 2, EB], FP32R, tag="act", name=f"u{b}"))
                xijT = lin_h(u, "wbt", "bb", False,
                             actp.tile([128, 2, EB], FP32R, tag="act",
                                       name=f"xij{b}"))
                h = xcn_sb
                for li, (wn, bn, rl) in enumerate((
                        ("w1t", "b1", True), ("w2t", "b2", True),
                        ("w3t", "b3", False))):
                    h = lin_h(h, wn, bn, rl,
                              actp.tile([128, 2, EB], FP32R, tag="act",
                                        name=f"h{b}_{li}"))
                nc.vector.tensor_scalar_mul(h[:], h[:], beta_sb[:])
                nc.vector.tensor_add(h[:], h[:], xijT[:])
                po = pop.tile([1, EB], FP32, tag="po")
                for k in range(2):
                    nc.tensor.matmul(po[:], wlt_sb[:, k, :], h[:, k, :],
                                     start=(k == 0), stop=(k == 1))
                nc.scalar.activation(out_sb[:, b * EB:(b + 1) * EB],
                                     po[:], AF.Identity, bias=bl_sb[:])

            # ---- main loop: gather -> AND -> xcn^T matmul ------------
            for b in range(NB):
                px = [pxp.tile([128, EB], FP32, tag="px", name=f"px{b}_{fh}")
                      for fh in range(2)]
                xv = None
                for q in range(NSPLIT):
                    nch = QELEM[q] // 128          # byte-chunks in tile
                    ac2 = QC2[q]                   # adjacency word-chunks
                    ab = []
                    for s in range(2):
                        gsl = slice((2 * b + s) * EB // 16,
                                    (2 * b + s + 1) * EB // 16)
                        t = gathp.tile([128, nch, EB], FP8, tag=f"g{q}",
                                       bufs=3, name=f"a{b}{q}{s}")
                        nc.gpsimd.dma_gather(
                            t[:], adjx[:, QOFF[q]:QOFF[q] + QELEM[q]],
                            idxg_sb[:, gsl], EB, EB,
                            elem_size=QELEM[q], elem_step=ROWB, transpose=True)
                        ab.append(t)
                    if q == NSPLIT - 1:
                        # bf16 x rows ride in the last 4 byte-chunks:
                        # feature (fh*128+p), edge i at [p, 2*ac2+2*fh+i//256,
                        # i%256] of the bf16 view -> [128, 2, 512]
                        xv = [t[:].bitcast(BF16)[:, 2 * ac2:2 * ac2 + 4, :]
                              .rearrange("p (f s) w -> p f (s w)", f=2)
                              for t in ab]
                    # cn = a0 AND a1 (binary fp8: bitwise AND == product),
                    # in place into a0, contiguous u16 for DVE 2x mode;
                    # split for AND->matmul overlap, adjacency chunks only
                    v0 = ab[0][:].bitcast(I16)
                    v1 = ab[1][:].bitcast(I16)
                    for hh in range(2):
                        csl = slice(hh * ac2, (hh + 1) * ac2)
                        nc.vector.tensor_tensor(
                            v0[:, csl, :], v0[:, csl, :], v1[:, csl, :],
                            mybir.AluOpType.bitwise_and)
                    # cn byte at (c2, par, i): flat = c2*1024 + 2i + par
                    va = (ab[0][:, 0:2 * ac2, :]
                          .rearrange("p cb j -> p (cb j)")
                          .rearrange("p (c i two) -> p c two i",
                                     c=ac2, two=2))
                    for w2 in range(ac2 // 2):
                        gc = GBASE[q] + 2 * w2
                        for par in range(2):
                            for fh in range(2):
                                if DOUBLE_ROW:
                                    nc.tensor.matmul(
                                        px[fh][:],
                                        x8t_sb[:, gc:gc + 2, par, fh, :],
                                        va[:, 2 * w2:2 * w2 + 2, par, :],
                                        start=(q == 0 and w2 == 0 and par == 0),
                                        stop=(q == NSPLIT - 1
                                              and w2 == ac2 // 2 - 1
                                              and par == 1),
                                        perf_mode=mybir.MatmulPerfMode.DoubleRow)
                                else:
                                    for t2 in range(2):
                                        nc.tensor.matmul(
                                            px[fh][:],
                                            x8t_sb[:, gc + t2, par, fh, :],
                                            va[:, 2 * w2 + t2, par, :],
                                            start=(q == 0 and w2 == 0
                                                   and par == 0 and t2 == 0),
                                            stop=(q == NSPLIT - 1
                                                  and w2 == ac2 // 2 - 1
                                                  and par == 1 and t2 == 1))
                xcn_sb = actp.tile([128, 2, EB], FP32R, tag="act",
                                   name=f"xcn{b}")
                for fh in range(2):
                    nc.scalar.activation(xcn_sb[:, fh, :], px[fh][:], AF.Copy)
                mlp_block(b, xcn_sb, xv[0], xv[1])

            nc.sync.dma_start(out_d[:], out_sb[:])

    nc.compile()
    return nc


def _wrap_idx(ids, num):
    """Pack indices for dma_gather: [128, num//16] int16, idx i at
    [i % 16, i // 16], replicated over the 8 groups of 16 partitions."""
    a = np.asarray(ids).astype(np.int16)
    w = a.reshape(num // 16, 16).T.copy()
    return np.ascontiguousarray(np.tile(w, (8, 1)))


def prepare_inputs(x, adj, edge, W1, b1, W2, b2, W3, b3, Wa, ba, Wb, bb,
                   Wl, bl, beta):
    x = np.asarray(x, np.float32)
    adj = np.asarray(adj, np.float32)
    edge = np.asarray(edge)

    # extended rows: [adj fp8 (10240B) | x bf16 (512B)]
    adjx8 = np.zeros((N, ROWB), np.uint8)
    adjx8[:, :N] = adj.astype(FP8_NP).view(np.uint8)
    adjx8[:, NPAD:] = np.ascontiguousarray(
        x.astype(BF16_NP)).view(np.uint8).reshape(N, 2 * D)
    adjx = adjx8.view(FP8_NP)

    # fp8 x table, permuted to the gather interleave:
    # x8t[p, gc, par, fh, f] = x8[gc*256 + 2p + par, fh*128 + f]
    x8 = np.zeros((NPAD, D), FP8_NP)
    x8[:N] = x.astype(FP8_NP)
    x8t = np.ascontiguousarray(
        x8.reshape(NGC, 128, 2, 2, 128)
        .transpose(1, 0, 2, 3, 4).reshape(128, -1))

    common = dict(
        adjx=adjx, x8t=x8t,
        wat=np.ascontiguousarray(np.asarray(Wa, np.float32).T),
        wbt=np.ascontiguousarray(np.asarray(Wb, np.float32).T),
        w1t=np.ascontiguousarray(np.asarray(W1, np.float32).T),
        w2t=np.ascontiguousarray(np.asarray(W2, np.float32).T),
        w3t=np.ascontiguousarray(np.asarray(W3, np.float32).T),
        wlt=np.ascontiguousarray(np.asarray(Wl, np.float32).T),
        ba=np.asarray(ba, np.float32).reshape(2, 128, 1),
        bb=np.asarray(bb, np.float32).reshape(2, 128, 1),
        b1=np.asarray(b1, np.float32).reshape(2, 128, 1),
        b2=np.asarray(b2, np.float32).reshape(2, 128, 1),
        b3=np.asarray(b3, np.float32).reshape(2, 128, 1),
        bl=np.asarray(bl, np.float32).reshape(1, 1),
        beta=np.full((128, 1), np.asarray(beta, np.float32).reshape(-1)[0],
                     np.float32),
    )
    in_maps = []
    for c in range(N_CORES):
        m = dict(common)
        gi = []
        for b in range(NB):
            sl = slice(c * EC + b * EB, c * EC + (b + 1) * EB)
            for s in range(2):
                gi.append(_wrap_idx(edge[sl, s], EB))
        m["idxg"] = np.ascontiguousarray(np.hstack(gi))
        in_maps.append(m)
    return in_maps


_CACHE = {}


def _get_program():
    if "nc" not in _CACHE:
        _CACHE["nc"] = build_program()
    return _CACHE["nc"]


def run(in_maps, **kw):
    nc = _get_program()
    return run_bass_kernel_spmd(nc, in_maps, list(range(N_CORES)), **kw)


def kernel(**inputs):
    in_maps = prepare_inputs(**inputs)
    res = run(in_maps)
    out = np.concatenate([res.results[c]["out"][0] for c in range(N_CORES)])
    return out.reshape(E, 1).astype(np.float32)



# revision 5
# speedup vs baseline: 1.4161x; 1.4161x over previous
"""CNLP (common-neighbor link prediction) kernel for Trainium2, 8 NeuronCores.

Reference computation (per query edge e = (i, j)):
    cn  = adj[i] * adj[j]                      # common-neighbor indicator [N]
    xcn = cn @ x                               # sum of common-neighbor feats
    xij = relu(x[i]*x[j] @ Wa.T + ba) @ Wb.T + bb
    hcn = (relu->relu->lin) 3-layer MLP on xcn
    out = (hcn * beta + xij) @ Wl.T + bl       # [E, 1]

Sharding: edges (E=8192) split 8 x 1024 across cores; adj/x/weights replicated.

Device strategy per core (1024 edges in 2 blocks of 512):
  - adj is binary -> BIT-PACKED host-side (10240 nodes -> 1280 bytes/row,
    8x less gather traffic than fp8).  Extended row: [packed 1280B | x bf16
    512B].  Per (block, src): one gpsimd dma_gather(transpose=True) for the
    packed part and one for the bf16 x part.
  - DVE ANDs the two packed rows (u16 2x mode), then EXPANDS bits to fp8
    bytes with 8 fused shift+mask tensor_scalar ops per block:
        OUT[p, m, c, e] = shift_m(cn_packed[p, c, e]) & 0x1010
    giving fp8 byte 0x10 (=2^-5) at node 2048c + 16p + 8par + m (par = byte
    within the u16 lane).  The arbitrary node permutation is absorbed into
    the host-permuted stationary x table, which is pre-scaled by 32 so
    2^-5 * 32x = x exactly.
  - Big matmul FLIPPED: stationary = permuted fp8 x table, moving = expanded
    cn slices; PSUM accumulates xcn^T feature-major [128f, 512e] directly.
    DoubleRow fp8 perf mode (2 k-tiles = adjacent m-planes) for 2x PE rate.
  - MLPs run feature-major with fp32 weights/activations read as float32r
    (fp22) -> 1 cycle/row.  xij path uses the gathered bf16 x rows.
"""

import numpy as np
import ml_dtypes

import concourse.bacc as bacc
import concourse.tile as tile
import concourse.mybir as mybir
from concourse.bass_utils import run_bass_kernel_spmd

BF16 = mybir.dt.bfloat16
FP32 = mybir.dt.float32
FP32R = mybir.dt.float32r
FP8 = mybir.dt.float8e4
I16 = mybir.dt.int16
AF = mybir.ActivationFunctionType
ALU = mybir.AluOpType
BF16_NP = ml_dtypes.bfloat16
FP8_NP = ml_dtypes.float8_e4m3

N_CORES = 8
N, E, D, H = 10000, 8192, 256, 256
NPAD = 10240                      # n padded to a multiple of 2048
EC = E // N_CORES                 # 1024 edges per core
EB = 512                          # edges per block
NB = EC // EB                     # 2 blocks
PKB = NPAD // 8                   # 1280 packed adjacency bytes per row
ROWB = PKB + 2 * D                # 1792 bytes per extended row
NC5 = PKB // 256                  # 5 u16 word-chunks of packed bits
XSCALE = 32.0                     # x table pre-scale (cn byte is 2^-5)


def build_program():
    nc = bacc.Bacc("TRN2", target_bir_lowering=False, debug=False,
                   enable_asserts=False, num_devices=N_CORES)

    adjx = nc.dram_tensor("adjx", [N, ROWB], FP8, kind="ExternalInput")
    # permuted+scaled fp8 x table, mp-major: [p][mp][c][par][fh][t][f]
    x8n_d = nc.dram_tensor("x8n", [128, 4 * NC5 * 2 * 2 * 2 * 128], FP8,
                           kind="ExternalInput")
    # gather idx (raw node ids), 512-idx wraps ordered (b, src)
    idxg_d = nc.dram_tensor("idxg", [128, NB * 2 * EB // 16], I16,
                            kind="ExternalInput")
    wts_d = {nm: nc.dram_tensor(nm, [D, H], FP32, kind="ExternalInput")
             for nm in ("wat", "wbt", "w1t", "w2t", "w3t")}
    wlt_d = nc.dram_tensor("wlt", [H, 1], FP32, kind="ExternalInput")
    bias_d = {nm: nc.dram_tensor(nm, [2, 128, 1], FP32, kind="ExternalInput")
              for nm in ("ba", "bb", "b1", "b2", "b3")}
    bl_d = nc.dram_tensor("bl", [1, 1], FP32, kind="ExternalInput")
    beta_d = nc.dram_tensor("beta", [128, 1], FP32, kind="ExternalInput")
    out_d = nc.dram_tensor("out", [1, EC], FP32, kind="ExternalOutput")

    with tile.TileContext(nc) as tc:
        with (
            tc.tile_pool(name="const", bufs=1) as constp,
            tc.tile_pool(name="gath", bufs=4) as gathp,
            tc.tile_pool(name="exp", bufs=3) as expp,
            tc.tile_pool(name="acts", bufs=6) as actp,
            tc.tile_pool(name="px", bufs=4, space="PSUM") as pxp,
            tc.tile_pool(name="pm", bufs=2, space="PSUM") as pmp,
            tc.tile_pool(name="po", bufs=2, space="PSUM") as pop,
        ):
            # ---- persistent loads -------------------------------------
            # idx tiles FIRST (gathers wait on them; HWDGE is FIFO)
            idxg_sb = constp.tile([128, NB * 2 * EB // 16], I16)
            nc.sync.dma_start(idxg_sb[:], idxg_d[:])

            # fp8 x table, split by mp quarter so early matmuls start sooner
            x8n_sb = constp.tile([128, 4, NC5, 2, 2, 2, 128], FP8)
            qsz = NC5 * 2 * 2 * 2 * 128
            for mp in range(4):
                nc.sync.dma_start(
                    x8n_sb[:, mp, :, :, :, :, :]
                    .rearrange("p c q f t g -> p (c q f t g)"),
                    x8n_d[:, mp * qsz:(mp + 1) * qsz])

            # weights: DMA fp32 staging -> scalar-copy to fp32r (the BIR
            # verifier requires fp32r matmul operands to be fp32r-rounded)
            w_sb = {}
            for nm, d in wts_d.items():
                stg = constp.tile([128, 2, H], FP32, tag="wstg", bufs=2,
                                  name=f"ws_{nm}")
                nc.sync.dma_start(stg[:], d[:].rearrange("(k p) h -> p k h", p=128))
                t = constp.tile([128, 2, H], FP32R, tag=f"w_{nm}")
                nc.scalar.activation(t[:], stg[:], AF.Copy)
                w_sb[nm] = t
            wlt_stg = constp.tile([128, 2, 1], FP32)
            nc.sync.dma_start(wlt_stg[:], wlt_d[:].rearrange("(k p) o -> p k o", p=128))
            wlt_sb = constp.tile([128, 2, 1], FP32R)
            nc.scalar.activation(wlt_sb[:], wlt_stg[:], AF.Copy)
            b_sb = {}
            for nm, d in bias_d.items():
                t = constp.tile([128, 2, 1], FP32, tag=f"b_{nm}")
                nc.sync.dma_start(t[:], d[:].rearrange("t p o -> p t o"))
                b_sb[nm] = t
            bl_sb = constp.tile([1, 1], FP32)
            nc.sync.dma_start(bl_sb[:], bl_d[:])
            beta_sb = constp.tile([128, 1], FP32)
            nc.sync.dma_start(beta_sb[:], beta_d[:])

            out_sb = constp.tile([1, EC], FP32)

            # MLP layer, feature-major fp32r (fp22 reads, 1 cyc/row), 512 edges
            def lin_h(src, wname, bname, relu, dst):
                w, bias = w_sb[wname], b_sb[bname]
                for t in range(2):
                    pm = pmp.tile([128, EB], FP32, tag="pm")
                    for k in range(2):
                        nc.tensor.matmul(
                            pm[:], w[:, k, t * 128:(t + 1) * 128],
                            src[:, k, :], start=(k == 0), stop=(k == 1))
                    dsl = dst[:, t, :]
                    if t % 2 == 0:
                        nc.scalar.activation(
                            dsl, pm[:], AF.Relu if relu else AF.Identity,
                            bias=bias[:, t, :])
                    elif relu:
                        nc.vector.tensor_scalar(
                            dsl, pm[:], bias[:, t, :], 0.0,
                            ALU.add, ALU.max)
                    else:
                        nc.vector.tensor_scalar_add(dsl, pm[:], bias[:, t, :])
                return dst

            def mlp_block(b, xcn_sb, xiT, xjT):
                pT = actp.tile([128, 2, EB], FP32R, tag="act")
                nc.vector.tensor_mul(pT[:], xiT, xjT)
                u = lin_h(pT, "wat", "ba", True,
                          actp.tile([128, 2, EB], FP32R, tag="act", name=f"u{b}"))
                xijT = lin_h(u, "wbt", "bb", False,
                             actp.tile([128, 2, EB], FP32R, tag="act",
                                       name=f"xij{b}"))
                h = xcn_sb
                for li, (wn, bn, rl) in enumerate((
                        ("w1t", "b1", True), ("w2t", "b2", True),
                        ("w3t", "b3", False))):
                    h = lin_h(h, wn, bn, rl,
                              actp.tile([128, 2, EB], FP32R, tag="act",
                                        name=f"h{b}_{li}"))
                nc.vector.tensor_scalar_mul(h[:], h[:], beta_sb[:])
                nc.vector.tensor_add(h[:], h[:], xijT[:])
                po = pop.tile([1, EB], FP32, tag="po")
                for k in range(2):
                    nc.tensor.matmul(po[:], wlt_sb[:, k, :], h[:, k, :],
                                     start=(k == 0), stop=(k == 1))
                nc.scalar.activation(out_sb[:, b * EB:(b + 1) * EB],
                                     po[:], AF.Identity, bias=bl_sb[:])

            # ---- main loop: gather -> AND -> expand -> xcn^T matmul ---
            for b in range(NB):
                ga = []     # packed adjacency gathers (src 0/1)
                gx = []     # bf16 x row gathers
                for s in range(2):
                    gsl = slice((2 * b + s) * EB // 16,
                                (2 * b + s + 1) * EB // 16)
                    t = gathp.tile([128, PKB // 128, EB], FP8, tag="gadj",
                                   bufs=4, name=f"a{b}{s}")
                    nc.gpsimd.dma_gather(
                        t[:], adjx[:, 0:PKB], idxg_sb[:, gsl], EB, EB,
                        elem_size=PKB, elem_step=ROWB, transpose=True)
                    ga.append(t)
                for s in range(2):
                    gsl = slice((2 * b + s) * EB // 16,
                                (2 * b + s + 1) * EB // 16)
                    t = gathp.tile([128, 2 * D // 128, EB], FP8, tag="gx",
                                   bufs=4, name=f"x{b}{s}")
                    nc.gpsimd.dma_gather(
                        t[:], adjx[:, PKB:ROWB], idxg_sb[:, gsl], EB, EB,
                        elem_size=2 * D, elem_step=ROWB, transpose=True)
                    gx.append(t)

                # cn_packed = a0 AND a1 (in place into a0), u16 2x mode.
                # u16 lane (c*512 + i) of partition p = packed word c*128+p
                # of edge i (16-bit granule gather transpose).
                v0 = ga[0][:].bitcast(I16)
                v1 = ga[1][:].bitcast(I16)
                nc.vector.tensor_tensor(v0, v0, v1, ALU.bitwise_and)
                v0f = v0.rearrange("p a b -> p (a b)")

                # bf16 x rows, feature-major [128, 2, 512]
                xv = [t[:].bitcast(BF16).rearrange("p (f s) w -> p f (s w)",
                                                   f=2) for t in gx]

                px = [pxp.tile([128, EB], FP32, tag="px", name=f"px{b}_{fh}")
                      for fh in range(2)]
                for mp in range(4):
                    # expand bit-planes 2mp, 2mp+1: fp8 byte 0x10 at
                    # node 2048c + 16p + 8par + m for set cn bits
                    om = expp.tile([128, 2, NC5, 2 * EB], FP8, tag="exp",
                                   name=f"om{b}_{mp}")
                    om16 = om[:].bitcast(I16)    # [128, 2, NC5, EB]
                    for t in range(2):
                        m = 2 * mp + t
                        dst = om16[:, t, :, :].rearrange("p c e -> p (c e)")
                        if m < 4:
                            nc.vector.tensor_scalar(
                                dst, v0f, 4 - m, 0x1010,
                                ALU.logical_shift_left, ALU.bitwise_and)
                        elif m == 4:
                            nc.vector.tensor_scalar(
                                dst, v0f, 0x1010, None,
                                ALU.bitwise_and)
                        else:
                            nc.vector.tensor_scalar(
                                dst, v0f, m - 4, 0x1010,
                                ALU.logical_shift_right, ALU.bitwise_and)
                    va = om[:]                   # [128, 2, NC5, 1024]
                    for c in range(NC5):
                        for par in range(2):
                            mov = (va[:, :, c, :]
                                   .rearrange("p t (i two) -> p t two i", two=2)
                                   [:, :, par, :])
                            for fh in range(2):
                                nc.tensor.matmul(
                                    px[fh][:],
                                    x8n_sb[:, mp, c, par, fh, :, :],
                                    mov,
                                    start=(mp == 0 and c == 0 and par == 0),
                                    stop=(mp == 3 and c == NC5 - 1
                                          and par == 1),
                                    perf_mode=mybir.MatmulPerfMode.DoubleRow)

                xcn_sb = actp.tile([128, 2, EB], FP32R, tag="act",
                                   name=f"xcn{b}")
                for fh in range(2):
                    nc.scalar.activation(xcn_sb[:, fh, :], px[fh][:], AF.Copy)
                mlp_block(b, xcn_sb, xv[0], xv[1])

            nc.sync.dma_start(out_d[:], out_sb[:])

    nc.compile()
    return nc


def _wrap_idx(ids, num):
    """Pack indices for dma_gather: [128, num//16] int16, idx i at
    [i % 16, i // 16], replicated over the 8 groups of 16 partitions."""
    a = np.asarray(ids).astype(np.int16)
    w = a.reshape(num // 16, 16).T.copy()
    return np.ascontiguousarray(np.tile(w, (8, 1)))


def prepare_inputs(x, adj, edge, W1, b1, W2, b2, W3, b3, Wa, ba, Wb, bb,
                   Wl, bl, beta):
    x = np.asarray(x, np.float32)
    adj = np.asarray(adj, np.float32)
    edge = np.asarray(edge)

    # extended rows: [packed adj bits (1280B) | x bf16 (512B)]
    adjp = np.zeros((N, NPAD), np.uint8)
    adjp[:, :N] = (adj != 0)
    adjx8 = np.zeros((N, ROWB), np.uint8)
    adjx8[:, :PKB] = np.packbits(adjp, axis=1, bitorder="little")
    adjx8[:, PKB:] = np.ascontiguousarray(
        x.astype(BF16_NP)).view(np.uint8).reshape(N, 2 * D)
    adjx = adjx8.view(FP8_NP)

    # permuted + scaled fp8 x table, mp-major:
    # x8n[p, mp, c, par, fh, t, f] = 32*x[2048c + 16p + 8par + 2mp + t,
    #                                     fh*128 + f]
    x8 = np.zeros((NPAD, D), FP8_NP)
    x8[:N] = np.clip(x * XSCALE, -224.0, 224.0).astype(FP8_NP)
    p_, mp_, c_, par_, t_ = np.meshgrid(
        np.arange(128), np.arange(4), np.arange(NC5), np.arange(2),
        np.arange(2), indexing="ij")
    nodes = 2048 * c_ + 16 * p_ + 8 * par_ + 2 * mp_ + t_
    # [p, mp, c, par, t, D] -> split f into (fh, f) -> [p, mp, c, par, fh, t, f]
    tbl = x8[nodes]                                    # [128,4,5,2,2,256]
    tbl = tbl.reshape(128, 4, NC5, 2, 2, 2, 128)       # t, fh, f
    tbl = np.ascontiguousarray(tbl.transpose(0, 1, 2, 3, 5, 4, 6))
    x8n = tbl.reshape(128, -1)

    common = dict(
        adjx=adjx, x8n=x8n,
        wat=np.ascontiguousarray(np.asarray(Wa, np.float32).T),
        wbt=np.ascontiguousarray(np.asarray(Wb, np.float32).T),
        w1t=np.ascontiguousarray(np.asarray(W1, np.float32).T),
        w2t=np.ascontiguousarray(np.asarray(W2, np.float32).T),
        w3t=np.ascontiguousarray(np.asarray(W3, np.float32).T),
        wlt=np.ascontiguousarray(np.asarray(Wl, np.float32).T),
        ba=np.asarray(ba, np.float32).reshape(2, 128, 1),
        bb=np.asarray(bb, np.float32).reshape(2, 128, 1),
        b1=np.asarray(b1, np.float32).reshape(2, 128, 1),
        b2=np.asarray(b2, np.float32).reshape(2, 128, 1),
        b3=np.asarray(b3, np.float32).reshape(2, 128, 1),
        bl=np.asarray(bl, np.float32).reshape(1, 1),
        beta=np.full((128, 1), np.asarray(beta, np.float32).reshape(-1)[0],
                     np.float32),
    )
    in_maps = []
    for c in range(N_CORES):
        m = dict(common)
        gi = []
        for b in range(NB):
            sl = slice(c * EC + b * EB, c * EC + (b + 1) * EB)
            for s in range(2):
                gi.append(_wrap_idx(edge[sl, s], EB))
        m["idxg"] = np.ascontiguousarray(np.hstack(gi))
        in_maps.append(m)
    return in_maps


_CACHE = {}


def _get_program():
    if "nc" not in _CACHE:
        _CACHE["nc"] = build_program()
    return _CACHE["nc"]


def run(in_maps, **kw):
    nc = _get_program()
    return run_bass_kernel_spmd(nc, in_maps, list(range(N_CORES)), **kw)


def kernel(**inputs):
    in_maps = prepare_inputs(**inputs)
    res = run(in_maps)
    out = np.concatenate([res.results[c]["out"][0] for c in range(N_CORES)])
    return out.reshape(E, 1).astype(np.float32)


# revision 7
# speedup vs baseline: 1.4288x; 1.0090x over previous
"""CNLP (common-neighbor link prediction) kernel for Trainium2, 8 NeuronCores.

Reference computation (per query edge e = (i, j)):
    cn  = adj[i] * adj[j]                      # common-neighbor indicator [N]
    xcn = cn @ x                               # sum of common-neighbor feats
    xij = relu(x[i]*x[j] @ Wa.T + ba) @ Wb.T + bb
    hcn = (relu->relu->lin) 3-layer MLP on xcn
    out = (hcn * beta + xij) @ Wl.T + bl       # [E, 1]

Sharding: edges (E=8192) split 8 x 1024 across cores; adj/x/weights replicated.

Device strategy per core (1024 edges in 2 blocks of 512):
  - adj is binary -> BIT-PACKED host-side (10240 nodes -> 1280 bytes/row,
    8x less gather traffic than fp8).  Extended row: [packed 1280B | x bf16
    512B].  Per (block, src): one gpsimd dma_gather(transpose=True) for the
    packed part and one for the bf16 x part.
  - DVE ANDs the two packed rows (u16 2x mode), then EXPANDS bits to fp8
    bytes with 8 fused shift+mask tensor_scalar ops per block:
        OUT[p, m, c, e] = shift_m(cn_packed[p, c, e]) & 0x1010
    giving fp8 byte 0x10 (=2^-5) at node 2048c + 16p + 8par + m (par = byte
    within the u16 lane).  The arbitrary node permutation is absorbed into
    the host-permuted stationary x table, which is pre-scaled by 32 so
    2^-5 * 32x = x exactly.
  - Big matmul FLIPPED: stationary = permuted fp8 x table, moving = expanded
    cn slices; PSUM accumulates xcn^T feature-major [128f, 512e] directly.
    DoubleRow fp8 perf mode (2 k-tiles = adjacent m-planes) for 2x PE rate.
  - MLPs run feature-major with fp32 weights/activations read as float32r
    (fp22) -> 1 cycle/row.  xij path uses the gathered bf16 x rows.
"""

import numpy as np
import ml_dtypes

import concourse.bacc as bacc
import concourse.tile as tile
import concourse.mybir as mybir
from concourse.bass_utils import run_bass_kernel_spmd

BF16 = mybir.dt.bfloat16
FP32 = mybir.dt.float32
FP32R = mybir.dt.float32r
FP8 = mybir.dt.float8e4
I16 = mybir.dt.int16
AF = mybir.ActivationFunctionType
ALU = mybir.AluOpType
BF16_NP = ml_dtypes.bfloat16
FP8_NP = ml_dtypes.float8_e4m3

N_CORES = 8
N, E, D, H = 10000, 8192, 256, 256
NPAD = 10240                      # n padded to a multiple of 2048
EC = E // N_CORES                 # 1024 edges per core
EB = 512                          # edges per block
NB = EC // EB                     # 2 blocks
PKB = NPAD // 8                   # 1280 packed adjacency bytes per row
ROWB = PKB + 2 * D                # 1792 bytes per extended row
NC5 = PKB // 256                  # 5 u16 word-chunks of packed bits
XSCALE = 32.0                     # x table pre-scale (cn byte is 2^-5)


def build_program():
    nc = bacc.Bacc("TRN2", target_bir_lowering=False, debug=False,
                   enable_asserts=False, num_devices=N_CORES)

    adjx = nc.dram_tensor("adjx", [N, ROWB], FP8, kind="ExternalInput")
    # permuted+scaled fp8 x table, mp-major: [p][mp][c][par][fh][t][f]
    x8n_d = nc.dram_tensor("x8n", [128, 4 * NC5 * 2 * 2 * 2 * 128], FP8,
                           kind="ExternalInput")
    # gather idx (raw node ids), 512-idx wraps ordered (b, src)
    idxg_d = nc.dram_tensor("idxg", [128, NB * 2 * EB // 16], I16,
                            kind="ExternalInput")
    wts_d = {nm: nc.dram_tensor(nm, [D, H], FP32, kind="ExternalInput")
             for nm in ("wat", "wbt", "w1t", "w2t", "w3t")}
    wlt_d = nc.dram_tensor("wlt", [H, 1], FP32, kind="ExternalInput")
    bias_d = {nm: nc.dram_tensor(nm, [2, 128, 1], FP32, kind="ExternalInput")
              for nm in ("ba", "bb", "b1", "b2", "b3")}
    bl_d = nc.dram_tensor("bl", [1, 1], FP32, kind="ExternalInput")
    beta_d = nc.dram_tensor("beta", [128, 1], FP32, kind="ExternalInput")
    out_d = nc.dram_tensor("out", [1, EC], FP32, kind="ExternalOutput")

    with tile.TileContext(nc) as tc:
        with (
            tc.tile_pool(name="const", bufs=1) as constp,
            tc.tile_pool(name="gath", bufs=4) as gathp,
            tc.tile_pool(name="exp", bufs=3) as expp,
            tc.tile_pool(name="acts", bufs=6) as actp,
            tc.tile_pool(name="px", bufs=4, space="PSUM") as pxp,
            tc.tile_pool(name="pm", bufs=2, space="PSUM") as pmp,
            tc.tile_pool(name="po", bufs=2, space="PSUM") as pop,
        ):
            # ---- persistent loads -------------------------------------
            # idx tiles FIRST (gathers wait on them; HWDGE is FIFO)
            idxg_sb = constp.tile([128, NB * 2 * EB // 16], I16)
            nc.sync.dma_start(idxg_sb[:], idxg_d[:])

            # gathers IMMEDIATELY after idx (gpsimd is otherwise idle and the
            # serial ~5us/gather issue cost is the pipeline's critical path).
            # One merged gather per (block, src): [packed adj 1280B | x bf16
            # 512B] = 1792B elements.
            gt = {}
            for b in range(NB):
                for s in range(2):
                    gsl = slice((2 * b + s) * EB // 16,
                                (2 * b + s + 1) * EB // 16)
                    t = gathp.tile([128, ROWB // 128, EB], FP8, tag="g",
                                   bufs=4, name=f"a{b}{s}")
                    nc.gpsimd.dma_gather(
                        t[:], adjx[:], idxg_sb[:, gsl], EB, EB,
                        elem_size=ROWB, transpose=True)
                    gt[(b, s)] = t

            # fp8 x table, split by mp quarter so early matmuls start sooner
            x8n_sb = constp.tile([128, 4, NC5, 2, 2, 2, 128], FP8)
            qsz = NC5 * 2 * 2 * 2 * 128
            for mp in range(4):
                nc.sync.dma_start(
                    x8n_sb[:, mp, :, :, :, :, :]
                    .rearrange("p c q f t g -> p (c q f t g)"),
                    x8n_d[:, mp * qsz:(mp + 1) * qsz])

            # weights: DMA fp32 staging -> scalar-copy to fp32r (the BIR
            # verifier requires fp32r matmul operands to be fp32r-rounded)
            w_sb = {}
            for nm, d in wts_d.items():
                stg = constp.tile([128, 2, H], FP32, tag="wstg", bufs=2,
                                  name=f"ws_{nm}")
                nc.sync.dma_start(stg[:], d[:].rearrange("(k p) h -> p k h", p=128))
                t = constp.tile([128, 2, H], FP32R, tag=f"w_{nm}")
                nc.scalar.activation(t[:], stg[:], AF.Copy)
                w_sb[nm] = t
            wlt_stg = constp.tile([128, 2, 1], FP32)
            nc.sync.dma_start(wlt_stg[:], wlt_d[:].rearrange("(k p) o -> p k o", p=128))
            wlt_sb = constp.tile([128, 2, 1], FP32R)
            nc.scalar.activation(wlt_sb[:], wlt_stg[:], AF.Copy)
            b_sb = {}
            for nm, d in bias_d.items():
                t = constp.tile([128, 2, 1], FP32, tag=f"b_{nm}")
                nc.sync.dma_start(t[:], d[:].rearrange("t p o -> p t o"))
                b_sb[nm] = t
            bl_sb = constp.tile([1, 1], FP32)
            nc.sync.dma_start(bl_sb[:], bl_d[:])
            beta_sb = constp.tile([128, 1], FP32)
            nc.sync.dma_start(beta_sb[:], beta_d[:])

            out_sb = constp.tile([1, EC], FP32)

            # MLP layer, feature-major fp32r (fp22 reads, 1 cyc/row), 512 edges
            def lin_h(src, wname, bname, relu, dst):
                w, bias = w_sb[wname], b_sb[bname]
                for t in range(2):
                    pm = pmp.tile([128, EB], FP32, tag="pm")
                    for k in range(2):
                        nc.tensor.matmul(
                            pm[:], w[:, k, t * 128:(t + 1) * 128],
                            src[:, k, :], start=(k == 0), stop=(k == 1))
                    dsl = dst[:, t, :]
                    if t % 2 == 0:
                        nc.scalar.activation(
                            dsl, pm[:], AF.Relu if relu else AF.Identity,
                            bias=bias[:, t, :])
                    elif relu:
                        nc.vector.tensor_scalar(
                            dsl, pm[:], bias[:, t, :], 0.0,
                            ALU.add, ALU.max)
                    else:
                        nc.vector.tensor_scalar_add(dsl, pm[:], bias[:, t, :])
                return dst

            def mlp_block(b, xcn_sb, xiT, xjT):
                pT = actp.tile([128, 2, EB], FP32R, tag="act")
                nc.vector.tensor_mul(pT[:], xiT, xjT)
                u = lin_h(pT, "wat", "ba", True,
                          actp.tile([128, 2, EB], FP32R, tag="act", name=f"u{b}"))
                xijT = lin_h(u, "wbt", "bb", False,
                             actp.tile([128, 2, EB], FP32R, tag="act",
                                       name=f"xij{b}"))
                h = xcn_sb
                for li, (wn, bn, rl) in enumerate((
                        ("w1t", "b1", True), ("w2t", "b2", True),
                        ("w3t", "b3", False))):
                    h = lin_h(h, wn, bn, rl,
                              actp.tile([128, 2, EB], FP32R, tag="act",
                                        name=f"h{b}_{li}"))
                nc.vector.tensor_scalar_mul(h[:], h[:], beta_sb[:])
                nc.vector.tensor_add(h[:], h[:], xijT[:])
                po = pop.tile([1, EB], FP32, tag="po")
                for k in range(2):
                    nc.tensor.matmul(po[:], wlt_sb[:, k, :], h[:, k, :],
                                     start=(k == 0), stop=(k == 1))
                nc.scalar.activation(out_sb[:, b * EB:(b + 1) * EB],
                                     po[:], AF.Identity, bias=bl_sb[:])

            # ---- main loop: AND -> expand -> xcn^T matmul -------------
            for b in range(NB):
                ga = [gt[(b, 0)], gt[(b, 1)]]

                # cn_packed = a0 AND a1 (adj byte-chunks 0..9 only, in place
                # into a0), u16 2x mode.  u16 lane (c*512 + i) of partition p
                # = packed word c*128+p of edge i (16-bit granule transpose).
                v0 = ga[0][:].bitcast(I16)[:, 0:2 * NC5, :]
                v1 = ga[1][:].bitcast(I16)[:, 0:2 * NC5, :]
                nc.vector.tensor_tensor(v0, v0, v1, ALU.bitwise_and)
                v0f = v0.rearrange("p a b -> p (a b)")

                # bf16 x rows (byte-chunks 10..13), feature-major [128, 2, 512]
                xv = [t[:].bitcast(BF16)[:, 2 * NC5:2 * NC5 + 4, :]
                      .rearrange("p (f s) w -> p f (s w)", f=2) for t in ga]

                px = [pxp.tile([128, EB], FP32, tag="px", name=f"px{b}_{fh}")
                      for fh in range(2)]
                for mp in range(4):
                    # expand bit-planes 2mp, 2mp+1: fp8 byte 0x10 at
                    # node 2048c + 16p + 8par + m for set cn bits
                    om = expp.tile([128, 2, NC5, 2 * EB], FP8, tag="exp",
                                   name=f"om{b}_{mp}")
                    om16 = om[:].bitcast(I16)    # [128, 2, NC5, EB]
                    for t in range(2):
                        m = 2 * mp + t
                        dst = om16[:, t, :, :].rearrange("p c e -> p (c e)")
                        if m < 4:
                            nc.vector.tensor_scalar(
                                dst, v0f, 4 - m, 0x1010,
                                ALU.logical_shift_left, ALU.bitwise_and)
                        elif m == 4:
                            nc.vector.tensor_scalar(
                                dst, v0f, 0x1010, None,
                                ALU.bitwise_and)
                        else:
                            nc.vector.tensor_scalar(
                                dst, v0f, m - 4, 0x1010,
                                ALU.logical_shift_right, ALU.bitwise_and)
                    va = om[:]                   # [128, 2, NC5, 1024]
                    for c in range(NC5):
                        for par in range(2):
                            mov = (va[:, :, c, :]
                                   .rearrange("p t (i two) -> p t two i", two=2)
                                   [:, :, par, :])
                            for fh in range(2):
                                nc.tensor.matmul(
                                    px[fh][:],
                                    x8n_sb[:, mp, c, par, fh, :, :],
                                    mov,
                                    start=(mp == 0 and c == 0 and par == 0),
                                    stop=(mp == 3 and c == NC5 - 1
                                          and par == 1),
                                    perf_mode=mybir.MatmulPerfMode.DoubleRow)

                xcn_sb = actp.tile([128, 2, EB], FP32R, tag="act",
                                   name=f"xcn{b}")
                for fh in range(2):
                    nc.scalar.activation(xcn_sb[:, fh, :], px[fh][:], AF.Copy)
                mlp_block(b, xcn_sb, xv[0], xv[1])

            nc.sync.dma_start(out_d[:], out_sb[:])

    nc.compile()
    return nc


def _wrap_idx(ids, num):
    """Pack indices for dma_gather: [128, num//16] int16, idx i at
    [i % 16, i // 16], replicated over the 8 groups of 16 partitions."""
    a = np.asarray(ids).astype(np.int16)
    w = a.reshape(num // 16, 16).T.copy()
    return np.ascontiguousarray(np.tile(w, (8, 1)))


def prepare_inputs(x, adj, edge, W1, b1, W2, b2, W3, b3, Wa, ba, Wb, bb,
                   Wl, bl, beta):
    x = np.asarray(x, np.float32)
    adj = np.asarray(adj, np.float32)
    edge = np.asarray(edge)

    # extended rows: [packed adj bits (1280B) | x bf16 (512B)]
    adjp = np.zeros((N, NPAD), np.uint8)
    adjp[:, :N] = (adj != 0)
    adjx8 = np.zeros((N, ROWB), np.uint8)
    adjx8[:, :PKB] = np.packbits(adjp, axis=1, bitorder="little")
    adjx8[:, PKB:] = np.ascontiguousarray(
        x.astype(BF16_NP)).view(np.uint8).reshape(N, 2 * D)
    adjx = adjx8.view(FP8_NP)

    # permuted + scaled fp8 x table, mp-major:
    # x8n[p, mp, c, par, fh, t, f] = 32*x[2048c + 16p + 8par + 2mp + t,
    #                                     fh*128 + f]
    x8 = np.zeros((NPAD, D), FP8_NP)
    x8[:N] = np.clip(x * XSCALE, -224.0, 224.0).astype(FP8_NP)
    p_, mp_, c_, par_, t_ = np.meshgrid(
        np.arange(128), np.arange(4), np.arange(NC5), np.arange(2),
        np.arange(2), indexing="ij")
    nodes = 2048 * c_ + 16 * p_ + 8 * par_ + 2 * mp_ + t_
    # [p, mp, c, par, t, D] -> split f into (fh, f) -> [p, mp, c, par, fh, t, f]
    tbl = x8[nodes]                                    # [128,4,5,2,2,256]
    tbl = tbl.reshape(128, 4, NC5, 2, 2, 2, 128)       # t, fh, f
    tbl = np.ascontiguousarray(tbl.transpose(0, 1, 2, 3, 5, 4, 6))
    x8n = tbl.reshape(128, -1)

    common = dict(
        adjx=adjx, x8n=x8n,
        wat=np.ascontiguousarray(np.asarray(Wa, np.float32).T),
        wbt=np.ascontiguousarray(np.asarray(Wb, np.float32).T),
        w1t=np.ascontiguousarray(np.asarray(W1, np.float32).T),
        w2t=np.ascontiguousarray(np.asarray(W2, np.float32).T),
        w3t=np.ascontiguousarray(np.asarray(W3, np.float32).T),
        wlt=np.ascontiguousarray(np.asarray(Wl, np.float32).T),
        ba=np.asarray(ba, np.float32).reshape(2, 128, 1),
        bb=np.asarray(bb, np.float32).reshape(2, 128, 1),
        b1=np.asarray(b1, np.float32).reshape(2, 128, 1),
        b2=np.asarray(b2, np.float32).reshape(2, 128, 1),
        b3=np.asarray(b3, np.float32).reshape(2, 128, 1),
        bl=np.asarray(bl, np.float32).reshape(1, 1),
        beta=np.full((128, 1), np.asarray(beta, np.float32).reshape(-1)[0],
                     np.float32),
    )
    in_maps = []
    for c in range(N_CORES):
        m = dict(common)
        gi = []
        for b in range(NB):
            sl = slice(c * EC + b * EB, c * EC + (b + 1) * EB)
            for s in range(2):
                gi.append(_wrap_idx(edge[sl, s], EB))
        m["idxg"] = np.ascontiguousarray(np.hstack(gi))
        in_maps.append(m)
    return in_maps


_CACHE = {}


def _get_program():
    if "nc" not in _CACHE:
        _CACHE["nc"] = build_program()
    return _CACHE["nc"]


def run(in_maps, **kw):
    nc = _get_program()
    return run_bass_kernel_spmd(nc, in_maps, list(range(N_CORES)), **kw)


def kernel(**inputs):
    in_maps = prepare_inputs(**inputs)
    res = run(in_maps)
    out = np.concatenate([res.results[c]["out"][0] for c in range(N_CORES)])
    return out.reshape(E, 1).astype(np.float32)


# revision 12
# speedup vs baseline: 1.4321x; 1.0023x over previous
"""CNLP (common-neighbor link prediction) kernel for Trainium2, 8 NeuronCores.

Reference computation (per query edge e = (i, j)):
    cn  = adj[i] * adj[j]                      # common-neighbor indicator [N]
    xcn = cn @ x                               # sum of common-neighbor feats
    xij = relu(x[i]*x[j] @ Wa.T + ba) @ Wb.T + bb
    hcn = (relu->relu->lin) 3-layer MLP on xcn
    out = (hcn * beta + xij) @ Wl.T + bl       # [E, 1]

Sharding: edges (E=8192) split 8 x 1024 across cores; adj/x/weights replicated.

Device strategy per core (1024 edges in 2 blocks of 512):
  - adj is binary -> BIT-PACKED host-side (10240 nodes -> 1280 bytes/row,
    8x less gather traffic than fp8).  Extended row: [packed 1280B | x bf16
    512B].  Per (block, src): one gpsimd dma_gather(transpose=True) for the
    packed part and one for the bf16 x part.
  - DVE ANDs the two packed rows (u16 2x mode), then EXPANDS bits to fp8
    bytes with 8 fused shift+mask tensor_scalar ops per block:
        OUT[p, m, c, e] = shift_m(cn_packed[p, c, e]) & 0x1010
    giving fp8 byte 0x10 (=2^-5) at node 2048c + 16p + 8par + m (par = byte
    within the u16 lane).  The arbitrary node permutation is absorbed into
    the host-permuted stationary x table, which is pre-scaled by 32 so
    2^-5 * 32x = x exactly.
  - Big matmul FLIPPED: stationary = permuted fp8 x table, moving = expanded
    cn slices; PSUM accumulates xcn^T feature-major [128f, 512e] directly.
    DoubleRow fp8 perf mode (2 k-tiles = adjacent m-planes) for 2x PE rate.
  - MLPs run feature-major with fp32 weights/activations read as float32r
    (fp22) -> 1 cycle/row.  xij path uses the gathered bf16 x rows.
"""

import numpy as np
import ml_dtypes

import concourse.bacc as bacc
import concourse.tile as tile
import concourse.mybir as mybir
from concourse.bass_utils import run_bass_kernel_spmd

BF16 = mybir.dt.bfloat16
FP32 = mybir.dt.float32
FP32R = mybir.dt.float32r
FP8 = mybir.dt.float8e4
I16 = mybir.dt.int16
AF = mybir.ActivationFunctionType
ALU = mybir.AluOpType
BF16_NP = ml_dtypes.bfloat16
FP8_NP = ml_dtypes.float8_e4m3

N_CORES = 8
N, E, D, H = 10000, 8192, 256, 256
NPAD = 10240                      # n padded to a multiple of 2048
EC = E // N_CORES                 # 1024 edges per core
EB = 512                          # edges per block
NB = EC // EB                     # 2 blocks
PKB = NPAD // 8                   # 1280 packed adjacency bytes per row
ROWB = PKB + 2 * D                # 1792 bytes per extended row
NC5 = PKB // 256                  # 5 u16 word-chunks of packed bits
XSCALE = 32.0                     # x table pre-scale (cn byte is 2^-5)


def build_program():
    nc = bacc.Bacc("TRN2", target_bir_lowering=False, debug=False,
                   enable_asserts=False, num_devices=N_CORES)

    adjx = nc.dram_tensor("adjx", [N, ROWB], FP8, kind="ExternalInput")
    # permuted+scaled fp8 x table, mp-major: [p][mp][c][par][fh][t][f]
    x8n_d = nc.dram_tensor("x8n", [128, 4 * NC5 * 2 * 2 * 2 * 128], FP8,
                           kind="ExternalInput")
    # gather idx (raw node ids), 512-idx wraps ordered (b, src)
    idxg_d = nc.dram_tensor("idxg", [128, NB * 2 * EB // 16], I16,
                            kind="ExternalInput")
    # all weights/biases consolidated into ONE load (the Sync engine
    # programs each dma_start serially at ~650ns; 19 of them stall startup)
    wpack_d = nc.dram_tensor("wpack", [128, 2576], FP32, kind="ExternalInput")
    out_d = nc.dram_tensor("out", [1, EC], FP32, kind="ExternalOutput")

    with tile.TileContext(nc) as tc:
        with (
            tc.tile_pool(name="const", bufs=1) as constp,
            tc.tile_pool(name="gath", bufs=4) as gathp,
            tc.tile_pool(name="exp", bufs=3) as expp,
            tc.tile_pool(name="acts", bufs=6) as actp,
            tc.tile_pool(name="px", bufs=4, space="PSUM") as pxp,
            tc.tile_pool(name="pm", bufs=2, space="PSUM") as pmp,
            tc.tile_pool(name="po", bufs=2, space="PSUM") as pop,
        ):
            # ---- persistent loads -------------------------------------
            # idx tiles FIRST (gathers wait on them; HWDGE is FIFO)
            idxg_sb = constp.tile([128, NB * 2 * EB // 16], I16)
            nc.sync.dma_start(idxg_sb[:], idxg_d[:])

            # gathers IMMEDIATELY after idx (gpsimd is otherwise idle and the
            # serial ~5us/gather issue cost is the pipeline's critical path).
            # One merged gather per (block, src): [packed adj 1280B | x bf16
            # 512B] = 1792B elements.
            gt = {}
            for b in range(NB):
                for s in range(2):
                    gsl = slice((2 * b + s) * EB // 16,
                                (2 * b + s + 1) * EB // 16)
                    t = gathp.tile([128, ROWB // 128, EB], FP8, tag="g",
                                   bufs=4, name=f"a{b}{s}")
                    nc.gpsimd.dma_gather(
                        t[:], adjx[:], idxg_sb[:, gsl], EB, EB,
                        elem_size=ROWB, transpose=True)
                    gt[(b, s)] = t

            # fp8 x table, split by mp quarter so early matmuls start sooner
            x8n_sb = constp.tile([128, 4, NC5, 2, 2, 2, 128], FP8)
            qsz = NC5 * 2 * 2 * 2 * 128
            for mp in range(4):
                nc.sync.dma_start(
                    x8n_sb[:, mp, :, :, :, :, :]
                    .rearrange("p c q f t g -> p (c q f t g)"),
                    x8n_d[:, mp * qsz:(mp + 1) * qsz])

            # weights: one packed DMA -> scalar-copy to fp32r (the BIR
            # verifier requires fp32r matmul operands to be fp32r-rounded)
            wpack = constp.tile([128, 2576], FP32)
            nc.sync.dma_start(wpack[:], wpack_d[:])
            w_sb = {}
            for i, nm in enumerate(("wat", "wbt", "w1t", "w2t", "w3t")):
                t = constp.tile([128, 2, H], FP32R, tag=f"w_{nm}")
                nc.scalar.activation(
                    t[:], wpack[:, i * 512:(i + 1) * 512]
                    .rearrange("p (k h) -> p k h", k=2), AF.Copy)
                w_sb[nm] = t
            wlt_sb = constp.tile([128, 2, 1], FP32R)
            nc.scalar.activation(
                wlt_sb[:], wpack[:, 2560:2562]
                .rearrange("p (k o) -> p k o", k=2), AF.Copy)
            b_sb = {}
            for i, nm in enumerate(("ba", "bb", "b1", "b2", "b3")):
                b_sb[nm] = (wpack[:, 2562 + 2 * i:2564 + 2 * i]
                            .rearrange("p (k o) -> p k o", k=2))
            bl_sb = wpack[0:1, 2572:2573]
            beta_sb = wpack[:, 2573:2574]

            out_sb = constp.tile([1, EC], FP32)

            # MLP layer, feature-major fp32r (fp22 reads, 1 cyc/row), 512 edges
            def lin_h(src, wname, bname, relu, dst):
                w, bias = w_sb[wname], b_sb[bname]
                for t in range(2):
                    pm = pmp.tile([128, EB], FP32, tag="pm")
                    for k in range(2):
                        nc.tensor.matmul(
                            pm[:], w[:, k, t * 128:(t + 1) * 128],
                            src[:, k, :], start=(k == 0), stop=(k == 1))
                    dsl = dst[:, t, :]
                    if t % 2 == 0:
                        nc.scalar.activation(
                            dsl, pm[:], AF.Relu if relu else AF.Identity,
                            bias=bias[:, t, :])
                    elif relu:
                        nc.vector.tensor_scalar(
                            dsl, pm[:], bias[:, t, :], 0.0,
                            ALU.add, ALU.max)
                    else:
                        nc.vector.tensor_scalar_add(dsl, pm[:], bias[:, t, :])
                return dst

            def mlp_block(b, xcn_sb, xiT, xjT):
                pT = actp.tile([128, 2, EB], FP32R, tag="act")
                nc.vector.tensor_mul(pT[:], xiT, xjT)
                u = lin_h(pT, "wat", "ba", True,
                          actp.tile([128, 2, EB], FP32R, tag="act", name=f"u{b}"))
                xijT = lin_h(u, "wbt", "bb", False,
                             actp.tile([128, 2, EB], FP32R, tag="act",
                                       name=f"xij{b}"))
                h = xcn_sb
                for li, (wn, bn, rl) in enumerate((
                        ("w1t", "b1", True), ("w2t", "b2", True),
                        ("w3t", "b3", False))):
                    h = lin_h(h, wn, bn, rl,
                              actp.tile([128, 2, EB], FP32R, tag="act",
                                        name=f"h{b}_{li}"))
                nc.vector.tensor_scalar_mul(h[:], h[:], beta_sb)
                nc.vector.tensor_add(h[:], h[:], xijT[:])
                po = pop.tile([1, EB], FP32, tag="po")
                for k in range(2):
                    nc.tensor.matmul(po[:], wlt_sb[:, k, :], h[:, k, :],
                                     start=(k == 0), stop=(k == 1))
                nc.scalar.activation(out_sb[:, b * EB:(b + 1) * EB],
                                     po[:], AF.Identity, bias=bl_sb)

            # ---- main loop: AND -> expand -> xcn^T matmul -------------
            for b in range(NB):
                ga = [gt[(b, 0)], gt[(b, 1)]]

                # cn_packed = a0 AND a1 (adj byte-chunks 0..9 only, in place
                # into a0), u16 2x mode.  u16 lane (c*512 + i) of partition p
                # = packed word c*128+p of edge i (16-bit granule transpose).
                v0 = ga[0][:].bitcast(I16)[:, 0:2 * NC5, :]
                v1 = ga[1][:].bitcast(I16)[:, 0:2 * NC5, :]
                nc.vector.tensor_tensor(v0, v0, v1, ALU.bitwise_and)
                v0f = v0.rearrange("p a b -> p (a b)")

                # bf16 x rows (byte-chunks 10..13), feature-major [128, 2, 512]
                xv = [t[:].bitcast(BF16)[:, 2 * NC5:2 * NC5 + 4, :]
                      .rearrange("p (f s) w -> p f (s w)", f=2) for t in ga]

                px = [pxp.tile([128, EB], FP32, tag="px", name=f"px{b}_{fh}")
                      for fh in range(2)]
                for mp in range(4):
                    # expand bit-planes 2mp, 2mp+1: fp8 byte 0x10 at
                    # node 2048c + 16p + 8par + m for set cn bits
                    om = expp.tile([128, 2, NC5, 2 * EB], FP8, tag="exp",
                                   name=f"om{b}_{mp}")
                    om16 = om[:].bitcast(I16)    # [128, 2, NC5, EB]
                    for t in range(2):
                        m = 2 * mp + t
                        dst = om16[:, t, :, :].rearrange("p c e -> p (c e)")
                        if m < 4:
                            nc.vector.tensor_scalar(
                                dst, v0f, 4 - m, 0x1010,
                                ALU.logical_shift_left, ALU.bitwise_and)
                        elif m == 4:
                            nc.vector.tensor_scalar(
                                dst, v0f, 0x1010, None,
                                ALU.bitwise_and)
                        else:
                            nc.vector.tensor_scalar(
                                dst, v0f, m - 4, 0x1010,
                                ALU.logical_shift_right, ALU.bitwise_and)
                    va = om[:]                   # [128, 2, NC5, 1024]
                    for c in range(NC5):
                        for par in range(2):
                            mov = (va[:, :, c, :]
                                   .rearrange("p t (i two) -> p t two i", two=2)
                                   [:, :, par, :])
                            for fh in range(2):
                                nc.tensor.matmul(
                                    px[fh][:],
                                    x8n_sb[:, mp, c, par, fh, :, :],
                                    mov,
                                    start=(mp == 0 and c == 0 and par == 0),
                                    stop=(mp == 3 and c == NC5 - 1
                                          and par == 1),
                                    perf_mode=mybir.MatmulPerfMode.DoubleRow)

                xcn_sb = actp.tile([128, 2, EB], FP32R, tag="act",
                                   name=f"xcn{b}")
                for fh in range(2):
                    nc.scalar.activation(xcn_sb[:, fh, :], px[fh][:], AF.Copy)
                mlp_block(b, xcn_sb, xv[0], xv[1])

            nc.sync.dma_start(out_d[:], out_sb[:])

    nc.compile()
    return nc


def _wrap_idx(ids, num):
    """Pack indices for dma_gather: [128, num//16] int16, idx i at
    [i % 16, i // 16], replicated over the 8 groups of 16 partitions."""
    a = np.asarray(ids).astype(np.int16)
    w = a.reshape(num // 16, 16).T.copy()
    return np.ascontiguousarray(np.tile(w, (8, 1)))


def prepare_inputs(x, adj, edge, W1, b1, W2, b2, W3, b3, Wa, ba, Wb, bb,
                   Wl, bl, beta):
    x = np.asarray(x, np.float32)
    adj = np.asarray(adj, np.float32)
    edge = np.asarray(edge)

    # extended rows: [packed adj bits (1280B) | x bf16 (512B)]
    adjp = np.zeros((N, NPAD), np.uint8)
    adjp[:, :N] = (adj != 0)
    adjx8 = np.zeros((N, ROWB), np.uint8)
    adjx8[:, :PKB] = np.packbits(adjp, axis=1, bitorder="little")
    adjx8[:, PKB:] = np.ascontiguousarray(
        x.astype(BF16_NP)).view(np.uint8).reshape(N, 2 * D)
    adjx = adjx8.view(FP8_NP)

    # permuted + scaled fp8 x table, mp-major:
    # x8n[p, mp, c, par, fh, t, f] = 32*x[2048c + 16p + 8par + 2mp + t,
    #                                     fh*128 + f]
    x8 = np.zeros((NPAD, D), FP8_NP)
    x8[:N] = np.clip(x * XSCALE, -224.0, 224.0).astype(FP8_NP)
    p_, mp_, c_, par_, t_ = np.meshgrid(
        np.arange(128), np.arange(4), np.arange(NC5), np.arange(2),
        np.arange(2), indexing="ij")
    nodes = 2048 * c_ + 16 * p_ + 8 * par_ + 2 * mp_ + t_
    # [p, mp, c, par, t, D] -> split f into (fh, f) -> [p, mp, c, par, fh, t, f]
    tbl = x8[nodes]                                    # [128,4,5,2,2,256]
    tbl = tbl.reshape(128, 4, NC5, 2, 2, 2, 128)       # t, fh, f
    tbl = np.ascontiguousarray(tbl.transpose(0, 1, 2, 3, 5, 4, 6))
    x8n = tbl.reshape(128, -1)

    # packed weights: per partition p (fp32 elems):
    #   [5 x (k,h)=2x256 transposed weights][wlt 2][5 x bias 2][bl][beta][pad]
    wpack = np.zeros((128, 2576), np.float32)
    for i, W in enumerate((Wa, Wb, W1, W2, W3)):
        wt = np.asarray(W, np.float32).T.reshape(2, 128, H)   # [k, p, h]
        wpack[:, i * 512:(i + 1) * 512] = wt.transpose(1, 0, 2).reshape(128, 512)
    wpack[:, 2560:2562] = np.asarray(Wl, np.float32).T.reshape(2, 128).T
    for i, bv in enumerate((ba, bb, b1, b2, b3)):
        wpack[:, 2562 + 2 * i:2564 + 2 * i] = \
            np.asarray(bv, np.float32).reshape(2, 128).T
    wpack[:, 2572] = np.asarray(bl, np.float32).reshape(-1)[0]
    wpack[:, 2573] = np.asarray(beta, np.float32).reshape(-1)[0]

    common = dict(adjx=adjx, x8n=x8n, wpack=wpack)
    in_maps = []
    for c in range(N_CORES):
        m = dict(common)
        gi = []
        for b in range(NB):
            sl = slice(c * EC + b * EB, c * EC + (b + 1) * EB)
            for s in range(2):
                gi.append(_wrap_idx(edge[sl, s], EB))
        m["idxg"] = np.ascontiguousarray(np.hstack(gi))
        in_maps.append(m)
    return in_maps


_CACHE = {}


def _get_program():
    if "nc" not in _CACHE:
        _CACHE["nc"] = build_program()
    return _CACHE["nc"]


def run(in_maps, **kw):
    nc = _get_program()
    return run_bass_kernel_spmd(nc, in_maps, list(range(N_CORES)), **kw)


def kernel(**inputs):
    in_maps = prepare_inputs(**inputs)
    res = run(in_maps)
    out = np.concatenate([res.results[c]["out"][0] for c in range(N_CORES)])
    return out.reshape(E, 1).astype(np.float32)
